# revision 32
# baseline (speedup 1.0000x reference)
"""EncDec ConvLSTM kernel for 8 Trainium2 NeuronCores.

Sharding: 8 cores = 4 (batch) x 2 (spatial row-halves). Each core computes
its 32 output rows plus a shrinking redundant halo (exact: 53-s rows at
recurrent step s), so no cross-core communication is needed. Row-half 1
cores receive a vertically flipped image and ky-flipped conv weights, so a
single SPMD program serves all cores.

Conv3x3 maps to PE matmuls over pixels (N = rows*64 free dim, bf16).
Per 8-row tile the 4H=256 gate channels come from 2 M-tiles x 6
accumulating matmuls: 1 x-im2col (K=72, im2col prebuilt on host, one DMA
per step), 3 kx-pair taps (K=128, via hh = [h; h shifted 2 cols]),
1 mid-column pair (K=128, via hh2 = [h; h shifted 2 rows]) and 1 single
mid tap (K=64, read from hh lower half directly).

The pointwise epilogue is software-pipelined with a 1-tile skew (phase1 =
gate activations + c update inputs, phase2 = tanh(c), h write, state
fan-out) so no engine queue head-of-line blocks. tanh(g) is computed as
2*sigmoid(2g)-1 with g-weights doubled on the host, merging the o/g
activations into one 128-partition sigmoid. DMA queues are dedicated:
sync = cross-partition t1 move, scalar = the 3 contiguous state band
copies, gpsimd = x-im2col + y output.
"""

import os
import sys

import numpy as np

for _p in ("/opt/trn_rl_repo", "/root/.axon_site/_ro/trn_rl_repo"):
    if os.path.isdir(_p) and _p not in sys.path:
        sys.path.append(_p)

T = 10
F = 8
HD = 64
HS = 64
WS = 64
NCORES = 8
PW = 66  # padded grid width/height
NSTEPS = 2 * T
X2LEN = 57 * PW  # prebuilt im2col free length

# big packed weight tensor: 12 x 256-col gate blocks + 5 x 8-col out blocks
_GBLK = ["xe", "pe0", "pe1", "pe2", "me", "se",
         "xd", "pd0", "pd1", "pd2", "md", "sd"]
_OBLK = ["op0", "op1", "op2", "om", "os"]
_WCOLS = 256 * len(_GBLK) + 8 * len(_OBLK)

_CACHE = {}


def _regions():
    """Exact compute-region row counts per recurrent step s=1..NSTEPS."""
    return [min(HS, 53 - s) for s in range(1, NSTEPS + 1)]


def _build_program(use_bf16=True):
    from concourse import bacc, mybir, tile

    F32 = mybir.dt.float32
    MMDT = mybir.dt.bfloat16 if use_bf16 else mybir.dt.float32r
    ACT = mybir.ActivationFunctionType
    ALU = mybir.AluOpType

    nc = bacc.Bacc("TRN2", target_bir_lowering=False, debug=False,
                   num_devices=NCORES)

    x2_d = nc.dram_tensor("x2", [NSTEPS, 72, X2LEN], MMDT,
                          kind="ExternalInput").ap()
    h1hh_d = nc.dram_tensor("h1hh", [128, PW * PW], MMDT,
                            kind="ExternalInput").ap()
    c1_d = nc.dram_tensor("c1", [64, HS * 64], MMDT,
                          kind="ExternalInput").ap()
    wall_d = nc.dram_tensor("wall", [128, _WCOLS], MMDT,
                            kind="ExternalInput").ap()
    ball_d = nc.dram_tensor("ball", [128, 5], F32,
                            kind="ExternalInput").ap()
    y_d = nc.dram_tensor("y", [T, F, 32, WS], F32, kind="ExternalOutput").ap()

    regions = _regions()

    with tile.TileContext(nc) as tc:
        with tc.tile_pool(name="wpool", bufs=1) as wp, \
             tc.tile_pool(name="state", bufs=1) as stp, \
             tc.tile_pool(name="x2p", bufs=3) as x2p, \
             tc.tile_pool(name="gps", bufs=8, space="PSUM") as gps, \
             tc.tile_pool(name="fip", bufs=3) as fip, \
             tc.tile_pool(name="ogp", bufs=6) as ogp, \
             tc.tile_pool(name="t1p", bufs=3) as t1p, \
             tc.tile_pool(name="t1lp", bufs=5) as t1lp, \
             tc.tile_pool(name="thp", bufs=3) as thp, \
             tc.tile_pool(name="yyp", bufs=2) as yyp:

            def load_x2_pre(s, q):
                rp = regions[s - 1]
                ln = (rp - 1) * PW + 64
                x2 = x2p.tile([72, X2LEN], MMDT, tag="x2", name=f"x2pre{s}")
                q.dma_start(x2[:, 0:ln], x2_d[s - 1][:, 0:ln])
                return x2

            wall = wp.tile([128, _WCOLS], MMDT, tag="wall")
            half = 256 * 6
            ball = wp.tile([128, 5], F32, tag="ball")

            goff = {k: 256 * i for i, k in enumerate(_GBLK)}
            ooff = {k: 256 * len(_GBLK) + 8 * i for i, k in enumerate(_OBLK)}

            def gw(key, m, kdim=128):
                o = goff[key] + 128 * m
                return wall[0:kdim, o:o + 128]

            def ow(key, kdim=128):
                o = ooff[key]
                return wall[0:kdim, o:o + 8]

            sb_b = {("e", 0): ball[:, 0:1], ("e", 1): ball[:, 1:2],
                    ("d", 0): ball[:, 2:3], ("d", 1): ball[:, 3:4]}
            sb_o = ball[:, 4:5]

            # ---- persistent state ----
            hhA = stp.tile([128, PW * PW], MMDT, tag="hhA")
            hhB = stp.tile([128, PW * PW], MMDT, tag="hhB")
            h2A = stp.tile([128, PW * PW], MMDT, tag="h2A")
            h2B = stp.tile([128, PW * PW], MMDT, tag="h2B")
            c_t = stp.tile([64, HS * 64], F32, tag="c")

            def gv(t_):
                return t_[:].rearrange("p (r c) -> p r c", c=PW)

            # Host uploads only h1 itself (rows <= 52 are read at step 2);
            # the three shifted layouts derive on-device via SBUF copies.
            # Step-2 im2col splits across both HWDGE queues ahead of all.
            L53 = 53 * PW
            x2_cur = load_x2_pre(2, nc.sync)
            nc.scalar.dma_start(hhA[0:64, 0:L53], h1hh_d[0:64, 0:L53])
            # encoder weights split across both queues behind the above;
            # the decoder half loads during step 3
            nc.sync.dma_start(wall[:, 0:half // 2], wall_d[:, 0:half // 2])
            nc.scalar.dma_start(wall[:, half // 2:half],
                                wall_d[:, half // 2:half])
            nc.sync.dma_start(ball[:], ball_d[:])
            nc.gpsimd.dma_start(c_t[:], c1_d[:])  # casts bf16 -> fp32
            x2_nxt = load_x2_pre(3, nc.gpsimd)
            nc.scalar.dma_start(hhA[64:128, 0:L53 - 2], hhA[0:64, 2:L53])
            nc.sync.dma_start(h2A[0:64, 0:L53], hhA[0:64, 0:L53])
            nc.scalar.dma_start(h2A[64:128, 0:L53 - 2 * PW],
                                hhA[0:64, 2 * PW:L53])
            # Zero only the borders that are read but never written:
            # row 0 everywhere; cols 0 and 65 of the h (lower) halves.
            for t_ in (hhB, h2B):
                v = gv(t_)
                nc.vector.memset(v[0:128, 0:1, 0:PW], 0.0)
            v = gv(hhB)
            nc.vector.memset(v[0:64, 0:PW, 0:1], 0.0)
            nc.vector.memset(v[0:64, 0:PW, 65:66], 0.0)

            def load_x2(s):
                rp = regions[s - 1]
                ln = (rp - 1) * PW + 64
                x2 = x2p.tile([72, X2LEN], MMDT, tag="x2")
                nc.gpsimd.dma_start(x2[:, 0:ln], x2_d[s - 1][:, 0:ln])
                return x2

            def emit_outconv(s, hv, h2v):
                """relu(out conv + bias) for decoder step s; 4 row-blocks
                run concurrently in distinct PE column groups."""
                t_o = s - 1 - T
                psos = [gps.tile([128, 512], F32, tag="ps",
                                 name=f"pso{j}") for j in range(4)]
                yy = yyp.tile([128, 512], F32, tag="yy")
                for k in range(3):
                    for j in range(4):
                        r0 = 8 * j
                        nc.tensor.matmul(psos[j][32 * j:32 * j + 8, :],
                                         ow(f"op{k}"),
                                         hv[:, r0 + k:r0 + k + 8, 0:64],
                                         start=(k == 0), stop=False,
                                         tile_position=(0, 32 * j))
                for j in range(4):
                    r0 = 8 * j
                    nc.tensor.matmul(psos[j][32 * j:32 * j + 8, :], ow("om"),
                                     h2v[:, r0:r0 + 8, 1:65],
                                     start=False, stop=False,
                                     tile_position=(0, 32 * j))
                for j in range(4):
                    r0 = 8 * j
                    nc.tensor.matmul(psos[j][32 * j:32 * j + 8, :],
                                     ow("os", 64),
                                     hv[0:64, r0 + 1:r0 + 9, 1:65],
                                     start=False, stop=True,
                                     tile_position=(0, 32 * j))
                for j in range(4):
                    sl = slice(32 * j, 32 * j + 8)
                    if j < 4:  # relu on ACT: DVE carries tg now
                        nc.scalar.activation(yy[sl, :], psos[j][sl, :],
                                             ACT.Relu, bias=sb_o[sl])
                    else:
                        nc.vector.tensor_scalar(
                            out=yy[sl, :], in0=psos[j][sl, :],
                            scalar1=sb_o[sl], scalar2=0.0,
                            op0=ALU.add, op1=ALU.max)
                    nc.gpsimd.dma_start(
                        y_d[t_o, :, 8 * j:8 * j + 8, :],
                        yy[sl].rearrange("p (r c) -> p r c", c=64))

            def gate_phase1(s, ph, r0, rows, hv_r, h2v_r, x2v):
                """Gate matmuls + activations + c-update inputs."""
                N = rows * 64
                ps0 = gps.tile([128, N], F32, tag="ps")
                ps1 = gps.tile([128, N], F32, tag="ps")
                for m, ps in ((0, ps0), (1, ps1)):
                    nc.tensor.matmul(ps[:], gw("x" + ph, m, 72),
                                     x2v[0:72, r0:r0 + rows, 0:64],
                                     start=True, stop=False)
                    for k in range(3):
                        nc.tensor.matmul(
                            ps[:], gw(f"p{ph}{k}", m),
                            hv_r[:, r0 + k:r0 + k + rows, 0:64],
                            start=False, stop=False)
                    nc.tensor.matmul(ps[:], gw("m" + ph, m),
                                     h2v_r[:, r0:r0 + rows, 1:65],
                                     start=False, stop=False)
                    nc.tensor.matmul(
                        ps[:], gw("s" + ph, m, 64),
                        hv_r[0:64, r0 + 1:r0 + 1 + rows, 1:65],
                        start=False, stop=True)

                # M0=[f;i] M1=[o;2g] (g pre-act doubled via host weights)
                fi = fip.tile([128, N], F32, tag="fi")
                og = ogp.tile([128, N], F32, tag="og")
                nc.scalar.activation(fi[:], ps0[:], ACT.Sigmoid,
                                     bias=sb_b[(ph, 0)])
                nc.scalar.activation(og[:], ps1[:], ACT.Sigmoid,
                                     bias=sb_b[(ph, 1)])
                cs = c_t[:, r0 * 64:r0 * 64 + N]  # [64, N] contiguous
                nc.vector.tensor_mul(cs, cs, fi[0:64])  # c *= sig(f)
                # t1 = sig(i) * tanh(g);  tanh(g) = 2*sig(2g) - 1
                t1 = t1p.tile([128, N], F32, tag="t1")
                nc.vector.tensor_scalar(
                    out=t1[64:128], in0=og[64:128], scalar1=2.0, scalar2=1.0,
                    op0=ALU.mult, op1=ALU.subtract)
                nc.vector.tensor_mul(t1[64:128], t1[64:128], fi[64:128])
                # cross-partition move fused with the c accumulate (SWDGE)
                nc.gpsimd.dma_start(cs, t1[64:128],
                                    accum_op=ALU.add)
                return (s, r0, rows, og, t1)

            def gate_phase2(ctxs, h_w, h2_w, hv_w):
                """c += t1, tanh(c), h write, state band fan-out.

                Takes 1-2 consecutive same-step tiles: tanh(c) and the
                band copies run merged over the pair (fewer, larger ops).
                """
                r0a = ctxs[0][1]
                rows_tot = sum(c[2] for c in ctxs)
                Nt = rows_tot * 64
                th = thp.tile([64, Nt], F32, tag="th")
                nc.scalar.activation(th[:], c_t[:, r0a * 64:r0a * 64 + Nt],
                                     ACT.Tanh)
                off = 0
                for _, r0, rows, og, _ in ctxs:
                    N = rows * 64
                    # h = tanh(c) * sigmoid(o) -> write buffer
                    nc.vector.tensor_mul(
                        hv_w[0:64, r0 + 1:r0 + 1 + rows, 1:65],
                        th[:, off:off + N].rearrange("p (r c) -> p r c",
                                                     c=64),
                        og[0:64].rearrange("p (r c) -> p r c", c=64))
                    off += N
                # state fan-out: contiguous flat band copies over the pair
                b0 = (r0a + 1) * PW
                L = rows_tot * PW
                hwf, h2f = h_w[:], h2_w[:]
                # hh upper: h shifted +2 cols (junk in pad cols, never read)
                nc.sync.dma_start(hwf[64:128, b0:b0 + L - 2],
                                  hwf[0:64, b0 + 2:b0 + L])
                # hh2 lower: plain copy of h
                nc.sync.dma_start(h2f[0:64, b0:b0 + L],
                                  hwf[0:64, b0:b0 + L])
                # hh2 upper: h shifted +2 rows (band lands 2 rows up)
                if r0a == 0:
                    # dst band [b0-2PW, ...) clipped at 0: drop PW elems
                    nc.scalar.dma_start(h2f[64:128, 0:L - PW],
                                        hwf[0:64, b0 + PW:b0 + L])
                else:
                    nc.scalar.dma_start(
                        h2f[64:128, b0 - 2 * PW:b0 + L - 2 * PW],
                        hwf[0:64, b0:b0 + L])

            from collections import deque

            pend = deque()  # (ctx, h_w, h2_w, hv_w), pipeline skew

            def flush2():
                a = pend.popleft()
                gate_phase2([a[0]], *a[1:])

            for s in range(2, NSTEPS + 1):
                ph = "e" if s <= T else "d"
                rp = regions[s - 1]
                if s % 2 == 0:  # read buffers written at s-1
                    h_r, h2_r, h_w, h2_w = hhA, h2A, hhB, h2B
                else:
                    h_r, h2_r, h_w, h2_w = hhB, h2B, hhA, h2A
                hv_r, h2v_r, hv_w = gv(h_r), gv(h2_r), gv(h_w)

                if s == 3:
                    # decoder weights: first needed at step T+1, loads
                    # behind step 3's band copies on the sync queue
                    nc.sync.dma_start(wall[:, half:_WCOLS],
                                      wall_d[:, half:_WCOLS])
                x2v = x2_cur[:].rearrange("p (r c) -> p r c", c=PW)

                r0 = 0
                while r0 < rp:
                    rows = min(8, rp - r0)
                    ctx = gate_phase1(s, ph, r0, rows, hv_r, h2v_r, x2v)
                    pend.append((ctx, h_w, h2_w, hv_w))
                    if len(pend) > 1:
                        flush2()
                    if s > T + 1 and r0 == 8:
                        # prev decoder step's out conv: emitted behind two
                        # gate tiles so step s-1's epilogue tail has fully
                        # drained by the time the PE reaches it.
                        emit_outconv(s - 1, hv_r, h2v_r)
                    r0 += 8

                if s > T:
                    # bands with rows <= 33 feed the out conv of step s:
                    # emit their phase2 before the step ends.
                    while pend and pend[0][0][1] <= 32:
                        flush2()

                if s + 2 <= NSTEPS:
                    x2_cur, x2_nxt = x2_nxt, load_x2(s + 2)
                else:
                    x2_cur = x2_nxt

            while pend:
                flush2()
            # out conv for the final decoder step (NSTEPS even -> B bufs)
            emit_outconv(NSTEPS, gv(hhB), gv(h2B))

    nc.compile()
    return nc


def _prep_core_inputs(core, enc_in, dec_in, enc_W, enc_b, dec_W, dec_b,
                      out_W, out_b, use_bf16=True):
    import ml_dtypes
    mm_np = ml_dtypes.bfloat16 if use_bf16 else np.float32
    b, half = core // 2, core % 2
    # gate permutation: [f, i, o, g]
    perm = np.concatenate([np.arange(0, 128), np.arange(192, 256),
                           np.arange(128, 192)])

    def prep_x2(x):
        x = x[b]  # [T, F, 64, 64]
        if half:
            x = x[:, :, ::-1, :]
        xp = np.zeros((T, F, PW, PW), np.float32)
        xp[:, :, 1:65, 1:65] = x
        flat = xp.reshape(T, F, PW * PW)
        x2 = np.empty((T, 72, X2LEN), np.float32)
        for tap in range(9):
            sh = (tap // 3) * PW + (tap % 3)
            x2[:, tap * 8:(tap + 1) * 8, :] = flat[:, :, sh:sh + X2LEN]
        return x2

    def prep_gateW(W, bias):
        Wf = W[:, :, ::-1, :] if half else W
        Wp = np.ascontiguousarray(Wf[perm]).astype(np.float64)
        bp = bias[perm].astype(np.float64)
        # double the g gate so sigmoid(2g) gives tanh via 2s-1
        Wp[192:256] *= 2.0
        bp[192:256] *= 2.0
        lx = np.zeros((128, 256))
        lx[0:72] = Wp[:, :F].transpose(2, 3, 1, 0).reshape(72, 256)
        lp = [np.concatenate([Wp[:, F:, k, 0].T, Wp[:, F:, k, 2].T], axis=0)
              for k in range(3)]  # [128, 256]
        lm = np.concatenate([Wp[:, F:, 0, 1].T, Wp[:, F:, 2, 1].T],
                            axis=0)  # [128, 256]
        ls = np.zeros((128, 256))
        ls[0:64] = Wp[:, F:, 1, 1].T
        return (lx, lp, lm, ls, bp[0:128].reshape(128, 1),
                bp[128:256].reshape(128, 1))

    ex, ep, em, es, eb0, eb1 = prep_gateW(enc_W, enc_b)
    dx, dp, dm, ds, db0, db1 = prep_gateW(dec_W, dec_b)
    oWf = out_W[:, :, ::-1, :] if half else out_W
    opad = np.zeros((128, 8))
    blk = {"xe": ex, "xd": dx, "me": em, "md": dm, "se": es, "sd": ds}
    for k in range(3):
        blk[f"pe{k}"] = ep[k]
        blk[f"pd{k}"] = dp[k]
    op = {}
    for k in range(3):
        op[f"op{k}"] = np.concatenate(
            [oWf[:, :, k, 0].T, oWf[:, :, k, 2].T], axis=0)  # [128, 8]
    op["om"] = np.concatenate(
        [oWf[:, :, 0, 1].T, oWf[:, :, 2, 1].T], axis=0)  # [128, 8]
    os_ = opad.copy()
    os_[0:64] = oWf[:, :, 1, 1].T
    op["os"] = os_

    wall = np.concatenate([blk[k] for k in _GBLK] +
                          [op[k] for k in _OBLK], axis=1)
    assert wall.shape == (128, _WCOLS)

    ball = np.zeros((128, 5), np.float32)
    ball[:, 0:1] = eb0
    ball[:, 1:2] = eb1
    ball[:, 2:3] = db0
    ball[:, 3:4] = db1
    for j in range(4):
        ball[32 * j:32 * j + 8, 4] = out_b

    x2_all = np.concatenate([prep_x2(enc_in), prep_x2(dec_in)], axis=0)

    # ---- step 1 on the host: h1 = tanh(c1)*sig(o), c1 = sig(i)*tanh(g) ----
    # gates = conv(x_1) + b with h == 0; computed via the prebuilt im2col.
    Wf = enc_W[:, :, ::-1, :] if half else enc_W  # [256, 72, 3, 3]
    x1 = enc_in[b, 0]  # [F, 64, 64]
    if half:
        x1 = x1[:, ::-1, :]
    xp1 = np.zeros((F, PW, PW))
    xp1[:, 1:65, 1:65] = x1
    fl1 = xp1.reshape(F, PW * PW)
    G = np.zeros((256, 64, 64))
    for ky in range(3):
        for kx in range(3):
            sh = ky * PW + kx
            win = np.stack([fl1[:, sh + q * PW:sh + q * PW + 64]
                            for q in range(64)], axis=1)  # [F, 64, 64]
            G += np.einsum("of,fqc->oqc", Wf[:, :F, ky, kx], win)
    G += enc_b[:, None, None]
    sg = lambda v: 1.0 / (1.0 + np.exp(-v))
    c1 = sg(G[64:128]) * np.tanh(G[128:192])  # i * g  (reference order figo)
    h1 = np.tanh(c1) * sg(G[192:256])  # tanh(c) * o
    hp = np.zeros((64, PW, PW))
    hp[:, 1:65, 1:65] = h1
    flath = hp.reshape(64, PW * PW)
    h1hh = np.zeros((128, PW * PW))
    h1hh[0:64] = flath
    h1hh[64:128, :-2] = flath[:, 2:]
    c1f = np.ascontiguousarray(c1.reshape(64, 64 * 64).astype(mm_np))

    return {"x2": np.ascontiguousarray(x2_all.astype(mm_np)),
            "wall": np.ascontiguousarray(wall.astype(mm_np)),
            "ball": np.ascontiguousarray(ball),
            "h1hh": np.ascontiguousarray(h1hh.astype(mm_np)),
            "c1": c1f}


def _install_trace_hook():
    """Shim antenv.axon_hooks for NTFF profiling (dev only)."""
    import contextlib
    import ctypes
    import types

    so = "/opt/axon/libaxon_pjrt.so"
    if "antenv.axon_hooks" in sys.modules or not os.path.exists(so):
        return
    lib = ctypes.CDLL(so)
    if not hasattr(lib, "axon_start_nrt_profile"):
        return
    lib.axon_start_nrt_profile.argtypes = [ctypes.POINTER(ctypes.c_int64),
                                           ctypes.c_size_t]
    lib.axon_start_nrt_profile.restype = ctypes.c_int64
    lib.axon_stop_nrt_profile.argtypes = [ctypes.c_char_p]
    lib.axon_stop_nrt_profile.restype = ctypes.c_int64

    def _mk():
        @contextlib.contextmanager
        def _hook(output_dir, device_ids):
            import jax
            jax.devices()
            if device_ids:
                ids = (ctypes.c_int64 * len(device_ids))(*device_ids)
                rc = lib.axon_start_nrt_profile(ids, len(device_ids))
            else:
                rc = lib.axon_start_nrt_profile(None, 0)
            if rc != 0:
                raise RuntimeError(f"axon_start_nrt_profile rc={rc}")
            try:
                yield
            finally:
                lib.axon_stop_nrt_profile(str(output_dir).encode())
        return _hook

    mod = types.ModuleType("antenv.axon_hooks")
    mod.get_axon_ntff_profile_hook = _mk
    sys.modules["antenv.axon_hooks"] = mod


def kernel(enc_in, dec_in, enc_W, enc_b, dec_W, dec_b, out_W, out_b):
    from concourse.bass_utils import run_bass_kernel_spmd

    trace = os.environ.get("KERNEL_TRACE", "") == "1"
    if trace:
        _install_trace_hook()

    use_bf16 = os.environ.get("KERNEL_DTYPE", "bf16") != "f32r"
    if "nc" not in _CACHE:
        _CACHE["nc"] = _build_program(use_bf16)
    nc = _CACHE["nc"]

    args = (np.asarray(enc_in, np.float32), np.asarray(dec_in, np.float32),
            np.asarray(enc_W, np.float32), np.asarray(enc_b, np.float32),
            np.asarray(dec_W, np.float32), np.asarray(dec_b, np.float32),
            np.asarray(out_W, np.float32), np.asarray(out_b, np.float32))
    in_maps = [_prep_core_inputs(c, *args, use_bf16=use_bf16)
               for c in range(NCORES)]

    res = run_bass_kernel_spmd(nc, in_maps, list(range(NCORES)), trace=trace)
    if trace:
        _CACHE["exec_time_ns"] = res.exec_time_ns

    B = enc_in.shape[0]
    out = np.empty((B, T, F, HS, WS), np.float32)
    for c in range(NCORES):
        b, half = c // 2, c % 2
        yc = res.results[c]["y"]  # [T, F, 32, 64]
        if half:
            out[b, :, :, 32:64, :] = yc[:, :, ::-1, :]
        else:
            out[b, :, :, 0:32, :] = yc
    return out


# revision 33
# speedup vs baseline: 1.0118x; 1.0118x over previous
"""EncDec ConvLSTM kernel for 8 Trainium2 NeuronCores.

Sharding: 8 cores = 4 (batch) x 2 (spatial row-halves). Each core computes
its 32 output rows plus a shrinking redundant halo (exact: 53-s rows at
recurrent step s), so no cross-core communication is needed. Row-half 1
cores receive a vertically flipped image and ky-flipped conv weights, so a
single SPMD program serves all cores.

Conv3x3 maps to PE matmuls over pixels (N = rows*64 free dim, bf16).
Per 8-row tile the 4H=256 gate channels come from 2 M-tiles x 6
accumulating matmuls: 1 x-im2col (K=72, im2col prebuilt on host, one DMA
per step), 3 kx-pair taps (K=128, via hh = [h; h shifted 2 cols]),
1 mid-column pair (K=128, via hh2 = [h; h shifted 2 rows]) and 1 single
mid tap (K=64, read from hh lower half directly).

The pointwise epilogue is software-pipelined with a 1-tile skew (phase1 =
gate activations + c update inputs, phase2 = tanh(c), h write, state
fan-out) so no engine queue head-of-line blocks. tanh(g) is computed as
2*sigmoid(2g)-1 with g-weights doubled on the host, merging the o/g
activations into one 128-partition sigmoid. DMA queues are dedicated:
sync = cross-partition t1 move, scalar = the 3 contiguous state band
copies, gpsimd = x-im2col + y output.
"""

import os
import sys

import numpy as np

for _p in ("/opt/trn_rl_repo", "/root/.axon_site/_ro/trn_rl_repo"):
    if os.path.isdir(_p) and _p not in sys.path:
        sys.path.append(_p)

T = 10
F = 8
HD = 64
HS = 64
WS = 64
NCORES = 8
PW = 66  # padded grid width/height
NSTEPS = 2 * T
X2LEN = 57 * PW  # prebuilt im2col free length

# big packed weight tensor: 12 x 256-col gate blocks + 5 x 8-col out blocks
_GBLK = ["xe", "pe0", "pe1", "pe2", "me", "se",
         "xd", "pd0", "pd1", "pd2", "md", "sd"]
_OBLK = ["op0", "op1", "op2", "om", "os"]
_WCOLS = 256 * len(_GBLK) + 8 * len(_OBLK)

_CACHE = {}


def _regions():
    """Exact compute-region row counts per recurrent step s=1..NSTEPS."""
    return [min(HS, 53 - s) for s in range(1, NSTEPS + 1)]


def _build_program(use_bf16=True):
    from concourse import bacc, mybir, tile

    F32 = mybir.dt.float32
    MMDT = mybir.dt.bfloat16 if use_bf16 else mybir.dt.float32r
    ACT = mybir.ActivationFunctionType
    ALU = mybir.AluOpType

    nc = bacc.Bacc("TRN2", target_bir_lowering=False, debug=False,
                   num_devices=NCORES)

    x2_d = nc.dram_tensor("x2", [NSTEPS, 72, X2LEN], MMDT,
                          kind="ExternalInput").ap()
    h1hh_d = nc.dram_tensor("h1hh", [128, PW * PW], MMDT,
                            kind="ExternalInput").ap()
    c1_d = nc.dram_tensor("c1", [64, HS * 64], MMDT,
                          kind="ExternalInput").ap()
    wall_d = nc.dram_tensor("wall", [128, _WCOLS], MMDT,
                            kind="ExternalInput").ap()
    ball_d = nc.dram_tensor("ball", [128, 5], F32,
                            kind="ExternalInput").ap()
    y_d = nc.dram_tensor("y", [T, F, 32, WS], F32, kind="ExternalOutput").ap()

    regions = _regions()

    with tile.TileContext(nc) as tc:
        with tc.tile_pool(name="wpool", bufs=1) as wp, \
             tc.tile_pool(name="state", bufs=1) as stp, \
             tc.tile_pool(name="x2p", bufs=3) as x2p, \
             tc.tile_pool(name="gps", bufs=8, space="PSUM") as gps, \
             tc.tile_pool(name="fip", bufs=3) as fip, \
             tc.tile_pool(name="ogp", bufs=6) as ogp, \
             tc.tile_pool(name="t1p", bufs=3) as t1p, \
             tc.tile_pool(name="t1lp", bufs=5) as t1lp, \
             tc.tile_pool(name="thp", bufs=3) as thp, \
             tc.tile_pool(name="yyp", bufs=2) as yyp:

            def load_x2_pre(s, q):
                rp = regions[s - 1]
                ln = (rp - 1) * PW + 64
                x2 = x2p.tile([72, X2LEN], MMDT, tag="x2", name=f"x2pre{s}")
                q.dma_start(x2[:, 0:ln], x2_d[s - 1][:, 0:ln])
                return x2

            wall = wp.tile([128, _WCOLS], MMDT, tag="wall")
            half = 256 * 6
            ball = wp.tile([128, 5], F32, tag="ball")

            goff = {k: 256 * i for i, k in enumerate(_GBLK)}
            ooff = {k: 256 * len(_GBLK) + 8 * i for i, k in enumerate(_OBLK)}

            def gw(key, m, kdim=128):
                o = goff[key] + 128 * m
                return wall[0:kdim, o:o + 128]

            def ow(key, kdim=128):
                o = ooff[key]
                return wall[0:kdim, o:o + 8]

            sb_b = {("e", 0): ball[:, 0:1], ("e", 1): ball[:, 1:2],
                    ("d", 0): ball[:, 2:3], ("d", 1): ball[:, 3:4]}
            sb_o = ball[:, 4:5]

            # ---- persistent state ----
            hhA = stp.tile([128, PW * PW], MMDT, tag="hhA")
            hhB = stp.tile([128, PW * PW], MMDT, tag="hhB")
            h2A = stp.tile([128, PW * PW], MMDT, tag="h2A")
            h2B = stp.tile([128, PW * PW], MMDT, tag="h2B")
            c_t = stp.tile([64, HS * 64], F32, tag="c")

            def gv(t_):
                return t_[:].rearrange("p (r c) -> p r c", c=PW)

            # Host uploads only h1 itself (rows <= 52 are read at step 2);
            # the three shifted layouts derive on-device via SBUF copies.
            # Step-2 im2col splits across both HWDGE queues ahead of all.
            L53 = 53 * PW
            x2_cur = load_x2_pre(2, nc.sync)
            nc.scalar.dma_start(hhA[0:64, 0:L53], h1hh_d[0:64, 0:L53])
            # encoder weights split across both queues behind the above;
            # the decoder half loads during step 3
            nc.sync.dma_start(wall[:, 0:half // 2], wall_d[:, 0:half // 2])
            nc.scalar.dma_start(wall[:, half // 2:half],
                                wall_d[:, half // 2:half])
            nc.sync.dma_start(ball[:], ball_d[:])
            nc.gpsimd.dma_start(c_t[:], c1_d[:])  # casts bf16 -> fp32
            x2_nxt = load_x2_pre(3, nc.gpsimd)
            nc.scalar.dma_start(hhA[64:128, 0:L53 - 2], hhA[0:64, 2:L53])
            nc.sync.dma_start(h2A[0:64, 0:L53], hhA[0:64, 0:L53])
            nc.scalar.dma_start(h2A[64:128, 0:L53 - 2 * PW],
                                hhA[0:64, 2 * PW:L53])
            # Zero only the borders that are read but never written:
            # row 0 everywhere; cols 0 and 65 of the h (lower) halves.
            for t_ in (hhB, h2B):
                v = gv(t_)
                nc.vector.memset(v[0:128, 0:1, 0:PW], 0.0)
            v = gv(hhB)
            nc.vector.memset(v[0:64, 0:PW, 0:1], 0.0)
            nc.vector.memset(v[0:64, 0:PW, 65:66], 0.0)

            def load_x2(s):
                rp = regions[s - 1]
                ln = (rp - 1) * PW + 64
                x2 = x2p.tile([72, X2LEN], MMDT, tag="x2")
                nc.gpsimd.dma_start(x2[:, 0:ln], x2_d[s - 1][:, 0:ln])
                return x2

            def emit_outconv(s, hv, h2v, hwq=False):
                """relu(out conv + bias) for decoder step s; 4 row-blocks
                run concurrently in distinct PE column groups."""
                t_o = s - 1 - T
                psos = [gps.tile([128, 512], F32, tag="ps",
                                 name=f"pso{j}") for j in range(4)]
                yy = yyp.tile([128, 512], F32, tag="yy")
                for k in range(3):
                    for j in range(4):
                        r0 = 8 * j
                        nc.tensor.matmul(psos[j][32 * j:32 * j + 8, :],
                                         ow(f"op{k}"),
                                         hv[:, r0 + k:r0 + k + 8, 0:64],
                                         start=(k == 0), stop=False,
                                         tile_position=(0, 32 * j))
                for j in range(4):
                    r0 = 8 * j
                    nc.tensor.matmul(psos[j][32 * j:32 * j + 8, :], ow("om"),
                                     h2v[:, r0:r0 + 8, 1:65],
                                     start=False, stop=False,
                                     tile_position=(0, 32 * j))
                for j in range(4):
                    r0 = 8 * j
                    nc.tensor.matmul(psos[j][32 * j:32 * j + 8, :],
                                     ow("os", 64),
                                     hv[0:64, r0 + 1:r0 + 9, 1:65],
                                     start=False, stop=True,
                                     tile_position=(0, 32 * j))
                for j in range(4):
                    sl = slice(32 * j, 32 * j + 8)
                    if j < 4:  # relu on ACT: DVE carries tg now
                        nc.scalar.activation(yy[sl, :], psos[j][sl, :],
                                             ACT.Relu, bias=sb_o[sl])
                    else:
                        nc.vector.tensor_scalar(
                            out=yy[sl, :], in0=psos[j][sl, :],
                            scalar1=sb_o[sl], scalar2=0.0,
                            op0=ALU.add, op1=ALU.max)
                    yq = (nc.sync, nc.scalar)[j % 2] if hwq else nc.gpsimd
                    yq.dma_start(
                        y_d[t_o, :, 8 * j:8 * j + 8, :],
                        yy[sl].rearrange("p (r c) -> p r c", c=64))

            def gate_phase1(s, ph, r0, rows, hv_r, h2v_r, x2v):
                """Gate matmuls + activations + c-update inputs."""
                N = rows * 64
                ps0 = gps.tile([128, N], F32, tag="ps")
                ps1 = gps.tile([128, N], F32, tag="ps")
                for m, ps in ((0, ps0), (1, ps1)):
                    nc.tensor.matmul(ps[:], gw("x" + ph, m, 72),
                                     x2v[0:72, r0:r0 + rows, 0:64],
                                     start=True, stop=False)
                    for k in range(3):
                        nc.tensor.matmul(
                            ps[:], gw(f"p{ph}{k}", m),
                            hv_r[:, r0 + k:r0 + k + rows, 0:64],
                            start=False, stop=False)
                    nc.tensor.matmul(ps[:], gw("m" + ph, m),
                                     h2v_r[:, r0:r0 + rows, 1:65],
                                     start=False, stop=False)
                    nc.tensor.matmul(
                        ps[:], gw("s" + ph, m, 64),
                        hv_r[0:64, r0 + 1:r0 + 1 + rows, 1:65],
                        start=False, stop=True)

                # M0=[f;i] M1=[o;2g] (g pre-act doubled via host weights)
                fi = fip.tile([128, N], F32, tag="fi")
                og = ogp.tile([128, N], F32, tag="og")
                nc.scalar.activation(fi[:], ps0[:], ACT.Sigmoid,
                                     bias=sb_b[(ph, 0)])
                nc.scalar.activation(og[:], ps1[:], ACT.Sigmoid,
                                     bias=sb_b[(ph, 1)])
                cs = c_t[:, r0 * 64:r0 * 64 + N]  # [64, N] contiguous
                nc.vector.tensor_mul(cs, cs, fi[0:64])  # c *= sig(f)
                # t1 = sig(i) * tanh(g);  tanh(g) = 2*sig(2g) - 1
                t1 = t1p.tile([128, N], F32, tag="t1")
                nc.vector.tensor_scalar(
                    out=t1[64:128], in0=og[64:128], scalar1=2.0, scalar2=1.0,
                    op0=ALU.mult, op1=ALU.subtract)
                nc.vector.tensor_mul(t1[64:128], t1[64:128], fi[64:128])
                # cross-partition move fused with the c accumulate (SWDGE)
                nc.gpsimd.dma_start(cs, t1[64:128],
                                    accum_op=ALU.add)
                return (s, r0, rows, og, t1)

            def gate_phase2(ctxs, h_w, h2_w, hv_w):
                """c += t1, tanh(c), h write, state band fan-out.

                Takes 1-2 consecutive same-step tiles: tanh(c) and the
                band copies run merged over the pair (fewer, larger ops).
                """
                r0a = ctxs[0][1]
                rows_tot = sum(c[2] for c in ctxs)
                Nt = rows_tot * 64
                th = thp.tile([64, Nt], F32, tag="th")
                nc.scalar.activation(th[:], c_t[:, r0a * 64:r0a * 64 + Nt],
                                     ACT.Tanh)
                off = 0
                for _, r0, rows, og, _ in ctxs:
                    N = rows * 64
                    # h = tanh(c) * sigmoid(o) -> write buffer
                    nc.vector.tensor_mul(
                        hv_w[0:64, r0 + 1:r0 + 1 + rows, 1:65],
                        th[:, off:off + N].rearrange("p (r c) -> p r c",
                                                     c=64),
                        og[0:64].rearrange("p (r c) -> p r c", c=64))
                    off += N
                # state fan-out: contiguous flat band copies over the pair
                b0 = (r0a + 1) * PW
                L = rows_tot * PW
                hwf, h2f = h_w[:], h2_w[:]
                # hh upper: h shifted +2 cols (junk in pad cols, never read)
                nc.sync.dma_start(hwf[64:128, b0:b0 + L - 2],
                                  hwf[0:64, b0 + 2:b0 + L])
                # hh2 lower: plain copy of h
                nc.sync.dma_start(h2f[0:64, b0:b0 + L],
                                  hwf[0:64, b0:b0 + L])
                # hh2 upper: h shifted +2 rows (band lands 2 rows up)
                if r0a == 0:
                    # dst band [b0-2PW, ...) clipped at 0: drop PW elems
                    nc.scalar.dma_start(h2f[64:128, 0:L - PW],
                                        hwf[0:64, b0 + PW:b0 + L])
                else:
                    nc.scalar.dma_start(
                        h2f[64:128, b0 - 2 * PW:b0 + L - 2 * PW],
                        hwf[0:64, b0:b0 + L])

            from collections import deque

            pend = deque()  # (ctx, h_w, h2_w, hv_w), pipeline skew

            def flush2():
                a = pend.popleft()
                gate_phase2([a[0]], *a[1:])

            for s in range(2, NSTEPS + 1):
                ph = "e" if s <= T else "d"
                rp = regions[s - 1]
                if s % 2 == 0:  # read buffers written at s-1
                    h_r, h2_r, h_w, h2_w = hhA, h2A, hhB, h2B
                else:
                    h_r, h2_r, h_w, h2_w = hhB, h2B, hhA, h2A
                hv_r, h2v_r, hv_w = gv(h_r), gv(h2_r), gv(h_w)

                if s == 3:
                    # decoder weights: first needed at step T+1, loads
                    # behind step 3's band copies on the sync queue
                    nc.sync.dma_start(wall[:, half:_WCOLS],
                                      wall_d[:, half:_WCOLS])
                x2v = x2_cur[:].rearrange("p (r c) -> p r c", c=PW)

                r0 = 0
                while r0 < rp:
                    rows = min(8, rp - r0)
                    ctx = gate_phase1(s, ph, r0, rows, hv_r, h2v_r, x2v)
                    pend.append((ctx, h_w, h2_w, hv_w))
                    if len(pend) > 1:
                        flush2()
                    if s > T + 1 and r0 == 8:
                        # prev decoder step's out conv: emitted behind two
                        # gate tiles so step s-1's epilogue tail has fully
                        # drained by the time the PE reaches it.
                        emit_outconv(s - 1, hv_r, h2v_r)
                    r0 += 8

                if s > T:
                    # bands with rows <= 33 feed the out conv of step s:
                    # emit their phase2 before the step ends.
                    while pend and pend[0][0][1] <= 32:
                        flush2()

                if s + 2 <= NSTEPS:
                    x2_cur, x2_nxt = x2_nxt, load_x2(s + 2)
                else:
                    x2_cur = x2_nxt

            while pend:
                flush2()
            # out conv for the final decoder step (NSTEPS even -> B bufs);
            # y goes out on the now-idle HWDGE queues (SWDGE ~2us/DMA fixed)
            emit_outconv(NSTEPS, gv(hhB), gv(h2B), hwq=True)

    nc.compile()
    return nc


def _prep_core_inputs(core, enc_in, dec_in, enc_W, enc_b, dec_W, dec_b,
                      out_W, out_b, use_bf16=True):
    import ml_dtypes
    mm_np = ml_dtypes.bfloat16 if use_bf16 else np.float32
    b, half = core // 2, core % 2
    # gate permutation: [f, i, o, g]
    perm = np.concatenate([np.arange(0, 128), np.arange(192, 256),
                           np.arange(128, 192)])

    def prep_x2(x):
        x = x[b]  # [T, F, 64, 64]
        if half:
            x = x[:, :, ::-1, :]
        xp = np.zeros((T, F, PW, PW), np.float32)
        xp[:, :, 1:65, 1:65] = x
        flat = xp.reshape(T, F, PW * PW)
        x2 = np.empty((T, 72, X2LEN), np.float32)
        for tap in range(9):
            sh = (tap // 3) * PW + (tap % 3)
            x2[:, tap * 8:(tap + 1) * 8, :] = flat[:, :, sh:sh + X2LEN]
        return x2

    def prep_gateW(W, bias):
        Wf = W[:, :, ::-1, :] if half else W
        Wp = np.ascontiguousarray(Wf[perm]).astype(np.float64)
        bp = bias[perm].astype(np.float64)
        # double the g gate so sigmoid(2g) gives tanh via 2s-1
        Wp[192:256] *= 2.0
        bp[192:256] *= 2.0
        lx = np.zeros((128, 256))
        lx[0:72] = Wp[:, :F].transpose(2, 3, 1, 0).reshape(72, 256)
        lp = [np.concatenate([Wp[:, F:, k, 0].T, Wp[:, F:, k, 2].T], axis=0)
              for k in range(3)]  # [128, 256]
        lm = np.concatenate([Wp[:, F:, 0, 1].T, Wp[:, F:, 2, 1].T],
                            axis=0)  # [128, 256]
        ls = np.zeros((128, 256))
        ls[0:64] = Wp[:, F:, 1, 1].T
        return (lx, lp, lm, ls, bp[0:128].reshape(128, 1),
                bp[128:256].reshape(128, 1))

    ex, ep, em, es, eb0, eb1 = prep_gateW(enc_W, enc_b)
    dx, dp, dm, ds, db0, db1 = prep_gateW(dec_W, dec_b)
    oWf = out_W[:, :, ::-1, :] if half else out_W
    opad = np.zeros((128, 8))
    blk = {"xe": ex, "xd": dx, "me": em, "md": dm, "se": es, "sd": ds}
    for k in range(3):
        blk[f"pe{k}"] = ep[k]
        blk[f"pd{k}"] = dp[k]
    op = {}
    for k in range(3):
        op[f"op{k}"] = np.concatenate(
            [oWf[:, :, k, 0].T, oWf[:, :, k, 2].T], axis=0)  # [128, 8]
    op["om"] = np.concatenate(
        [oWf[:, :, 0, 1].T, oWf[:, :, 2, 1].T], axis=0)  # [128, 8]
    os_ = opad.copy()
    os_[0:64] = oWf[:, :, 1, 1].T
    op["os"] = os_

    wall = np.concatenate([blk[k] for k in _GBLK] +
                          [op[k] for k in _OBLK], axis=1)
    assert wall.shape == (128, _WCOLS)

    ball = np.zeros((128, 5), np.float32)
    ball[:, 0:1] = eb0
    ball[:, 1:2] = eb1
    ball[:, 2:3] = db0
    ball[:, 3:4] = db1
    for j in range(4):
        ball[32 * j:32 * j + 8, 4] = out_b

    x2_all = np.concatenate([prep_x2(enc_in), prep_x2(dec_in)], axis=0)

    # ---- step 1 on the host: h1 = tanh(c1)*sig(o), c1 = sig(i)*tanh(g) ----
    # gates = conv(x_1) + b with h == 0; computed via the prebuilt im2col.
    Wf = enc_W[:, :, ::-1, :] if half else enc_W  # [256, 72, 3, 3]
    x1 = enc_in[b, 0]  # [F, 64, 64]
    if half:
        x1 = x1[:, ::-1, :]
    xp1 = np.zeros((F, PW, PW))
    xp1[:, 1:65, 1:65] = x1
    fl1 = xp1.reshape(F, PW * PW)
    G = np.zeros((256, 64, 64))
    for ky in range(3):
        for kx in range(3):
            sh = ky * PW + kx
            win = np.stack([fl1[:, sh + q * PW:sh + q * PW + 64]
                            for q in range(64)], axis=1)  # [F, 64, 64]
            G += np.einsum("of,fqc->oqc", Wf[:, :F, ky, kx], win)
    G += enc_b[:, None, None]
    sg = lambda v: 1.0 / (1.0 + np.exp(-v))
    c1 = sg(G[64:128]) * np.tanh(G[128:192])  # i * g  (reference order figo)
    h1 = np.tanh(c1) * sg(G[192:256])  # tanh(c) * o
    hp = np.zeros((64, PW, PW))
    hp[:, 1:65, 1:65] = h1
    flath = hp.reshape(64, PW * PW)
    h1hh = np.zeros((128, PW * PW))
    h1hh[0:64] = flath
    h1hh[64:128, :-2] = flath[:, 2:]
    c1f = np.ascontiguousarray(c1.reshape(64, 64 * 64).astype(mm_np))

    return {"x2": np.ascontiguousarray(x2_all.astype(mm_np)),
            "wall": np.ascontiguousarray(wall.astype(mm_np)),
            "ball": np.ascontiguousarray(ball),
            "h1hh": np.ascontiguousarray(h1hh.astype(mm_np)),
            "c1": c1f}


def _install_trace_hook():
    """Shim antenv.axon_hooks for NTFF profiling (dev only)."""
    import contextlib
    import ctypes
    import types

    so = "/opt/axon/libaxon_pjrt.so"
    if "antenv.axon_hooks" in sys.modules or not os.path.exists(so):
        return
    lib = ctypes.CDLL(so)
    if not hasattr(lib, "axon_start_nrt_profile"):
        return
    lib.axon_start_nrt_profile.argtypes = [ctypes.POINTER(ctypes.c_int64),
                                           ctypes.c_size_t]
    lib.axon_start_nrt_profile.restype = ctypes.c_int64
    lib.axon_stop_nrt_profile.argtypes = [ctypes.c_char_p]
    lib.axon_stop_nrt_profile.restype = ctypes.c_int64

    def _mk():
        @contextlib.contextmanager
        def _hook(output_dir, device_ids):
            import jax
            jax.devices()
            if device_ids:
                ids = (ctypes.c_int64 * len(device_ids))(*device_ids)
                rc = lib.axon_start_nrt_profile(ids, len(device_ids))
            else:
                rc = lib.axon_start_nrt_profile(None, 0)
            if rc != 0:
                raise RuntimeError(f"axon_start_nrt_profile rc={rc}")
            try:
                yield
            finally:
                lib.axon_stop_nrt_profile(str(output_dir).encode())
        return _hook

    mod = types.ModuleType("antenv.axon_hooks")
    mod.get_axon_ntff_profile_hook = _mk
    sys.modules["antenv.axon_hooks"] = mod


def kernel(enc_in, dec_in, enc_W, enc_b, dec_W, dec_b, out_W, out_b):
    from concourse.bass_utils import run_bass_kernel_spmd

    trace = os.environ.get("KERNEL_TRACE", "") == "1"
    if trace:
        _install_trace_hook()

    use_bf16 = os.environ.get("KERNEL_DTYPE", "bf16") != "f32r"
    if "nc" not in _CACHE:
        _CACHE["nc"] = _build_program(use_bf16)
    nc = _CACHE["nc"]

    args = (np.asarray(enc_in, np.float32), np.asarray(dec_in, np.float32),
            np.asarray(enc_W, np.float32), np.asarray(enc_b, np.float32),
            np.asarray(dec_W, np.float32), np.asarray(dec_b, np.float32),
            np.asarray(out_W, np.float32), np.asarray(out_b, np.float32))
    in_maps = [_prep_core_inputs(c, *args, use_bf16=use_bf16)
               for c in range(NCORES)]

    res = run_bass_kernel_spmd(nc, in_maps, list(range(NCORES)), trace=trace)
    if trace:
        _CACHE["exec_time_ns"] = res.exec_time_ns

    B = enc_in.shape[0]
    out = np.empty((B, T, F, HS, WS), np.float32)
    for c in range(NCORES):
        b, half = c // 2, c % 2
        yc = res.results[c]["y"]  # [T, F, 32, 64]
        if half:
            out[b, :, :, 32:64, :] = yc[:, :, ::-1, :]
        else:
            out[b, :, :, 0:32, :] = yc
    return out
